# revision 12
# baseline (speedup 1.0000x reference)
"""GNN message-passing layer (LplsNorm + residual conv) on 8 Trainium2 cores.

Computation (reference, all f32):
    degree = A.sum(-1); ds = degree**-0.5
    mf  = f + ds[:,None] * (A @ (ds[:,None] * f))      # a_norm = ds A ds
    out = relu(mf @ W + b)

Distribution: A row-sharded over 8 cores ([1024, 8192] each), feature
replicated.

Per-core schedule (v5):
  - Single streaming pass over the A shard. Per [128, 2048] f32 chunk:
    DVE casts to bf16 + accumulates exact f32 row sums (degree), TensorE
    transposes the 16 [128,128] bf16 tiles via identity matmuls, ScalarE
    copies them out of PSUM as fp8(e4m3). The ENTIRE transposed shard
    stays SBUF-resident (8 MiB fp8 = 64 KiB/partition) - no DRAM scratch.
  - Degree AllGather is SPLIT in two: half 1 (m-tiles 0-3) is issued
    mid-phase-A and completes in its shadow; half 2 (m-tiles 4-7) is
    issued at phase-A end and hides under the first matmul half-pass,
    which only touches k-chunks whose ds came from half 1.
  - dsq = 64/sqrt(degree) (the x64 fp8 exponent boost is undone in the
    epilogue row scale dsown = sqrt(1/deg)/64).
  - X' = dsq * f in fp8, produced just-in-time from streamed f32 f
    chunks; f blocks are split so each matmul half-pass streams exactly
    the rows it needs.
  - Main matmul in fp8 DoubleRow mode (K=256 per instruction, 2x bf16
    rate), kc-pair-outer over 2 groups of 4 m-tiles (4 PSUM banks).
  - Epilogue per m-tile: mf = Y * dsown + f_res (DVE, bf16 out), mf @ W
    in bf16, bias via a K=1 f32r matmul, ACT relu. PSUM: o accumulator
    shares the phase-A transpose pool (2 bufs) so m-tiles pipeline.
"""

import numpy as np

import concourse.bass as bass
import concourse.mybir as mybir
import concourse.tile as tile
from concourse import bacc
from concourse import bass_utils
from concourse.masks import make_identity

N = 8192
D = 512
NCORES = 8
P = 128
R = N // NCORES          # rows per core: 1024
MT = R // P              # m-tiles per core: 8
KC = N // P              # k-chunks: 64
PAIRS = KC // 2          # DoubleRow k-pairs: 32
ACH = 2048               # A stream chunk width (f32 -> 1 MiB per DMA)
NACH = N // ACH          # stream chunks per row-block: 4
GPC = ACH // (4 * P)     # transpose groups (of 4 tiles) per stream chunk: 4
MTG = 4                  # m-tiles per matmul group (PSUM accumulators)

F32 = mybir.dt.float32
F32R = mybir.dt.float32r
BF16 = mybir.dt.bfloat16
FP8 = mybir.dt.float8e4

_NC_CACHE = {}


def _build():
    nc = bacc.Bacc("TRN2", target_bir_lowering=False, debug=False, num_devices=NCORES)

    a_d = nc.dram_tensor("a", [R, N], F32, kind="ExternalInput")
    f_d = nc.dram_tensor("f", [N, D], F32, kind="ExternalInput")
    fres_d = nc.dram_tensor("fres", [R, D], F32, kind="ExternalInput")
    w_d = nc.dram_tensor("w", [D, D], F32, kind="ExternalInput")
    b_d = nc.dram_tensor("bias", [1, D], F32R, kind="ExternalInput")
    out_d = nc.dram_tensor("out", [R, D], F32, kind="ExternalOutput")

    AX = mybir.AxisListType.X
    ALU = mybir.AluOpType
    ACT = mybir.ActivationFunctionType
    DR = mybir.MatmulPerfMode.DoubleRow

    with tile.TileContext(nc) as tc:
        with (
            tc.tile_pool(name="const", bufs=1) as constp,
            tc.tile_pool(name="deg", bufs=1) as degp,
            tc.tile_pool(name="astream", bufs=3) as astreamp,
            tc.tile_pool(name="small", bufs=2) as smallp,
            tc.tile_pool(name="atres", bufs=1) as atresp,
            tc.tile_pool(name="xp", bufs=1) as xpp,
            tc.tile_pool(name="fstream", bufs=4) as fstreamp,
            tc.tile_pool(name="epi", bufs=2) as epip,
            tc.tile_pool(name="mft", bufs=2) as mftp,
            tc.tile_pool(name="psA", bufs=2, space="PSUM") as psA,      # transposes + o
            tc.tile_pool(name="psY", bufs=MTG, space="PSUM") as psY,    # Y accumulators
            tc.tile_pool(name="psaux", bufs=1, space="PSUM") as psaux,  # small transposes
            tc.tile_pool(name="dram", bufs=1, space="DRAM") as dramp,
        ):
            # ---- constants ----
            identity = constp.tile([P, P], F32)
            make_identity(nc, identity[:])
            identity_bf = constp.tile([P, P], BF16)
            make_identity(nc, identity_bf[:])
            ones_row = constp.tile([1, P], F32)
            nc.gpsimd.memset(ones_row[:], 1.0)
            b_sb = constp.tile([1, D], F32R)
            nc.sync.dma_start(b_sb[:], b_d.ap())
            w_f32 = fstreamp.tile([P, 4 * D], F32, tag="fch")
            for wc in range(4):
                nc.sync.dma_start(
                    w_f32[:, wc * D : (wc + 1) * D], w_d.ap()[wc * P : (wc + 1) * P, :]
                )
            w_sb = constp.tile([P, 4 * D], BF16)  # w chunk wc at [:, wc*D:(wc+1)*D]
            nc.vector.tensor_copy(w_sb[:], w_f32[:])

            # fully resident transposed-A store, fp8:
            # k-chunk kc of m-tile mt at [:, (mt*KC + kc)*P : (mt*KC + kc + 1)*P]
            at_res = atresp.tile([P, MT * KC * P], FP8)
            cin1 = dramp.tile([MT // 2, P], F32)
            cin2 = dramp.tile([MT // 2, P], F32)
            cout1 = dramp.tile([KC // 2, P], F32)
            cout2 = dramp.tile([KC // 2, P], F32)

            # degree collective half h covers m-tiles [h*4, h*4+4) of every
            # core, i.e. global k-chunks g with g%8 in [h*4, h*4+4).
            def issue_half(h, cin, cout):
                aux = psaux.tile([P, 4 * P], F32, tag="aux")
                degT_ps = aux[0 : MT // 2, 0:P]
                nc.tensor.transpose(
                    degT_ps, degree_sb[:, h * 4 : h * 4 + 4], identity[:]
                )
                degT_sb = smallp.tile([MT // 2, P], F32, tag=f"degT{h}")
                nc.vector.tensor_copy(degT_sb[:], degT_ps)
                nc.sync.dma_start(cin[:], degT_sb[:])
                nc.gpsimd.collective_compute(
                    "AllGather",
                    ALU.bypass,
                    ins=[cin.opt()],
                    outs=[cout.opt()],
                    replica_groups=[list(range(NCORES))],
                )

            def consume_half(h, cout, dsq_half):
                # cout row r = degree of k-chunk (r//4)*8 + h*4 + r%4
                degall_sb = smallp.tile([KC // 2, P], F32, tag=f"degall{h}")
                nc.sync.dma_start(degall_sb[:], cout[:])
                aux2 = psaux.tile([P, 4 * P], F32, tag="aux")
                degallT_ps = aux2[0:P, 0 : KC // 2]
                nc.tensor.transpose(
                    degallT_ps, degall_sb[:], identity[: KC // 2, : KC // 2]
                )
                recip = smallp.tile([P, KC // 2], F32, tag=f"recip{h}")
                nc.vector.reciprocal(recip[:], degallT_ps)
                # dsq_half[p, c*4 + i] = 64 * ds[c*8 + h*4 + i], with x64 boost
                nc.scalar.activation(dsq_half[:], recip[:], ACT.Sqrt, scale=4096.0)

            def dsq_col(kc):
                h = (kc % 8) // 4
                col = (kc // 8) * 4 + (kc % 8) % 4
                return dsqs[h][:, col : col + 1]

            dsqs = [degp.tile([P, KC // 2], F32, name=f"dsq{h}") for h in range(2)]

            # ---- merged pass: degree + transpose-all ----
            degree_sb = degp.tile([P, MT], F32)  # col mt = degree of rows mt*128..
            for mt in range(MT):
                dcols = smallp.tile([P, NACH], F32, tag="dcols")
                for c in range(NACH):
                    ach = astreamp.tile([P, ACH], F32, tag="ach")
                    nc.sync.dma_start(
                        ach[:], a_d.ap()[mt * P : (mt + 1) * P, c * ACH : (c + 1) * ACH]
                    )
                    achb = astreamp.tile([P, ACH], BF16, tag="achb", bufs=2)
                    nc.vector.tensor_scalar(
                        achb[:],
                        ach[:],
                        1.0,
                        0.0,
                        op0=ALU.mult,
                        op1=ALU.add,
                        accum_out=dcols[:, c : c + 1],
                    )
                    for g in range(GPC):
                        kc0 = c * GPC * 4 + g * 4  # first k-chunk of this group
                        trp = psA.tile([P, 4 * P], F32, tag="trp")
                        for q in range(4):
                            nc.tensor.matmul(
                                trp[:, q * P : (q + 1) * P],
                                achb[:, (g * 4 + q) * P : (g * 4 + q + 1) * P],
                                identity_bf[:],
                            )
                        dst = at_res[:, (mt * KC + kc0) * P : (mt * KC + kc0 + 4) * P]
                        nc.scalar.activation(dst, trp[:], ACT.Copy)
                nc.vector.reduce_sum(degree_sb[:, mt : mt + 1], dcols[:], axis=AX)
                if mt == MT // 2 - 1:
                    issue_half(0, cin1, cout1)
            issue_half(1, cin2, cout2)
            consume_half(0, cout1, dsqs[0])

            # local ds of own rows, /64 to undo the fp8 boost
            recip8 = degp.tile([P, MT], F32)
            nc.vector.reciprocal(recip8[:], degree_sb[:])
            dsown = degp.tile([P, MT], F32)
            nc.scalar.activation(dsown[:], recip8[:], ACT.Sqrt, scale=1.0 / 4096.0)

            # X' = dsq * f in fp8; produced during mtg 0 below.
            xp_sb = xpp.tile([P, KC * D], FP8)  # chunk kc at [:, kc*D:(kc+1)*D]
            # f block fb covers k-chunks 4*fb..4*fb+3; fb = 2*c + h covers
            # the half-h chunks of core c's rows.
            f_blk = f_d.ap().rearrange("(b c p) d -> b p c d", c=4, p=P)

            # pair order: half-1 pairs (both kc have kc%8 < 4) first
            jhalf = [[4 * c + u + 2 * h for c in range(8) for u in range(2)]
                     for h in range(2)]

            # ---- main matmul: fp8 DoubleRow, kc-pair-outer, 2 groups of 4 m-tiles ----
            for mtg in range(MT // MTG):
                # prefetch residual rows for this group's epilogue
                ress = []
                for mi in range(MTG):
                    mt = mtg * MTG + mi
                    res = epip.tile([P, D], F32, tag="res", bufs=MTG)
                    nc.sync.dma_start(res[:], fres_d.ap()[mt * P : (mt + 1) * P, :])
                    ress.append(res)
                ys = [
                    psY.tile([P, D], F32, tag="y", name=f"y{mtg}_{i}")
                    for i in range(MTG)
                ]
                jorder = jhalf[0] + jhalf[1]
                for jn, j in enumerate(jorder):
                    if mtg == 0 and jn == PAIRS // 2:
                        # collective 2 has completed under the half-1 pass;
                        # emitting its consumption here keeps its cout DMA
                        # from head-of-line-blocking the half-1 f stream.
                        consume_half(1, cout2, dsqs[1])
                    if mtg == 0 and jn % 2 == 0:
                        # stream the f block feeding this pair + the next
                        fb = 2 * (j // 4) + (j % 4) // 2  # = 2*c + h
                        fch = fstreamp.tile([P, 4 * D], F32, tag="fch")
                        nc.sync.dma_start(
                            fch[:].rearrange("p (c d) -> p c d", c=4), f_blk[fb]
                        )
                        for t in range(4):
                            kc = 4 * fb + t
                            nc.vector.tensor_scalar_mul(
                                xp_sb[:, kc * D : (kc + 1) * D],
                                fch[:, t * D : (t + 1) * D],
                                dsq_col(kc),
                            )
                    rhs = xp_sb[:, (2 * j) * D : (2 * j + 2) * D].rearrange(
                        "p (two n) -> p two n", two=2
                    )
                    for mi in range(MTG):
                        mt = mtg * MTG + mi
                        lhsT = at_res[
                            :, (mt * KC + 2 * j) * P : (mt * KC + 2 * j + 2) * P
                        ].rearrange("p (two m) -> p two m", two=2)
                        nc.tensor.matmul(
                            ys[mi][:],
                            lhsT,
                            rhs,
                            start=(jn == 0),
                            stop=(jn == PAIRS - 1),
                            perf_mode=DR,
                        )
                # epilogue per m-tile in the group
                for mi in range(MTG):
                    mt = mtg * MTG + mi
                    mf = epip.tile([P, D], BF16, tag="mf")
                    nc.vector.scalar_tensor_tensor(
                        mf[:],
                        ys[mi][:],
                        dsown[:, mt : mt + 1],
                        ress[mi][:],
                        op0=ALU.mult,
                        op1=ALU.add,
                    )
                    o_ps = psA.tile([P, D], F32, tag="trp")
                    aux = psaux.tile([P, 4 * P], F32, tag="aux")
                    for wc in range(4):
                        mfT_ps = aux[:, wc * P : (wc + 1) * P]
                        nc.tensor.matmul(
                            mfT_ps, mf[:, wc * P : (wc + 1) * P], identity_bf[:]
                        )
                        mfT_sb = mftp.tile([P, P], BF16, tag="mfT")
                        nc.scalar.activation(mfT_sb[:], mfT_ps, ACT.Copy)
                        nc.tensor.matmul(
                            o_ps[:],
                            mfT_sb[:],
                            w_sb[:, wc * D : (wc + 1) * D],
                            start=(wc == 0),
                            stop=False,
                        )
                    nc.tensor.matmul(
                        o_ps[:], ones_row[:].bitcast(F32R), b_sb[:],
                        start=False, stop=True,
                    )
                    osb = epip.tile([P, D], F32, tag="osb")
                    nc.scalar.activation(osb[:], o_ps[:], ACT.Relu)
                    nc.sync.dma_start(out_d.ap()[mt * P : (mt + 1) * P, :], osb[:])

    nc.compile()
    return nc


def _get_nc():
    if "nc" not in _NC_CACHE:
        _NC_CACHE["nc"] = _build()
    return _NC_CACHE["nc"]


def run(inputs, trace=False, trace_kwargs=None):
    """Run the SPMD kernel; returns (full_output, BassKernelResults)."""
    a = np.ascontiguousarray(np.asarray(inputs["adjacency_matrix"], dtype=np.float32))
    f = np.ascontiguousarray(np.asarray(inputs["feature"], dtype=np.float32))
    w = np.ascontiguousarray(np.asarray(inputs["W"], dtype=np.float32))
    b = np.ascontiguousarray(np.asarray(inputs["b"], dtype=np.float32)).reshape(1, D)

    nc = _get_nc()
    in_maps = []
    for d in range(NCORES):
        rows = slice(d * R, (d + 1) * R)
        in_maps.append({"a": a[rows], "f": f, "fres": f[rows], "w": w, "bias": b})
    res = bass_utils.run_bass_kernel_spmd(
        nc,
        in_maps,
        core_ids=list(range(NCORES)),
        trace=trace,
        **(trace_kwargs or {}),
    )
    out = np.concatenate([r["out"] for r in res.results], axis=0)
    return out, res


def kernel(**inputs):
    out, _ = run(inputs, trace=False)
    return out


# revision 18
# speedup vs baseline: 1.1181x; 1.1181x over previous
"""GNN message-passing layer (LplsNorm + residual conv) on 8 Trainium2 cores.

Computation (reference, all f32):
    degree = A.sum(-1); ds = degree**-0.5
    mf  = f + ds[:,None] * (A @ (ds[:,None] * f))      # a_norm = ds A ds
    out = relu(mf @ W + b)

Distribution: A row-sharded over 8 cores ([1024, 8192] each), feature
replicated.

Per-core schedule (v5):
  - Single streaming pass over the A shard. Per [128, 2048] f32 chunk:
    DVE casts to bf16 + accumulates exact f32 row sums (degree), TensorE
    transposes the 16 [128,128] bf16 tiles via identity matmuls, ScalarE
    copies them out of PSUM as fp8(e4m3). The ENTIRE transposed shard
    stays SBUF-resident (8 MiB fp8 = 64 KiB/partition) - no DRAM scratch.
  - Degree AllGather is SPLIT in two: half 1 (m-tiles 0-3) is issued
    mid-phase-A and completes in its shadow; half 2 (m-tiles 4-7) is
    issued at phase-A end and hides under the first matmul half-pass,
    which only touches k-chunks whose ds came from half 1.
  - dsq = 64/sqrt(degree) (the x64 fp8 exponent boost is undone in the
    epilogue row scale dsown = sqrt(1/deg)/64).
  - X' = dsq * f in fp8, produced just-in-time from streamed f32 f
    chunks; f blocks are split so each matmul half-pass streams exactly
    the rows it needs.
  - Main matmul in fp8 DoubleRow mode (K=256 per instruction, 2x bf16
    rate), kc-pair-outer over 2 groups of 4 m-tiles (4 PSUM banks).
  - Epilogue per m-tile: mf = Y * dsown + f_res (DVE, bf16 out), mf @ W
    in bf16, bias via a K=1 f32r matmul, ACT relu. PSUM: o accumulator
    shares the phase-A transpose pool (2 bufs) so m-tiles pipeline.
"""

import numpy as np

import concourse.bass as bass
import concourse.mybir as mybir
import concourse.tile as tile
from concourse import bacc
from concourse import bass_utils
from concourse.masks import make_identity

N = 8192
D = 512
NCORES = 8
P = 128
R = N // NCORES          # rows per core: 1024
MT = R // P              # m-tiles per core: 8
KC = N // P              # k-chunks: 64
PAIRS = KC // 2          # DoubleRow k-pairs: 32
ACH = 2048               # A stream chunk width (f32 -> 1 MiB per DMA)
NACH = N // ACH          # stream chunks per row-block: 4
GPC = ACH // (4 * P)     # transpose groups (of 4 tiles) per stream chunk: 4
MTG = 4                  # m-tiles per matmul group (PSUM accumulators)

F32 = mybir.dt.float32
F32R = mybir.dt.float32r
BF16 = mybir.dt.bfloat16
FP8 = mybir.dt.float8e4

_NC_CACHE = {}


def _build():
    nc = bacc.Bacc("TRN2", target_bir_lowering=False, debug=False, num_devices=NCORES)

    a_d = nc.dram_tensor("a", [R, N], F32, kind="ExternalInput")
    f_d = nc.dram_tensor("f", [N, D], F32, kind="ExternalInput")
    fres_d = nc.dram_tensor("fres", [R, D], F32, kind="ExternalInput")
    w_d = nc.dram_tensor("w", [D, D], F32, kind="ExternalInput")
    b_d = nc.dram_tensor("bias", [1, D], F32R, kind="ExternalInput")
    out_d = nc.dram_tensor("out", [R, D], F32, kind="ExternalOutput")

    AX = mybir.AxisListType.X
    ALU = mybir.AluOpType
    ACT = mybir.ActivationFunctionType
    DR = mybir.MatmulPerfMode.DoubleRow

    with tile.TileContext(nc) as tc:
        with (
            tc.tile_pool(name="const", bufs=1) as constp,
            tc.tile_pool(name="deg", bufs=1) as degp,
            tc.tile_pool(name="astream", bufs=3) as astreamp,
            tc.tile_pool(name="small", bufs=2) as smallp,
            tc.tile_pool(name="atres", bufs=1) as atresp,
            tc.tile_pool(name="xp", bufs=1) as xpp,
            tc.tile_pool(name="fstream", bufs=5) as fstreamp,
            tc.tile_pool(name="epi", bufs=2) as epip,
            tc.tile_pool(name="mft", bufs=2) as mftp,
            tc.tile_pool(name="psA", bufs=2, space="PSUM") as psA,      # transposes + o
            tc.tile_pool(name="psY", bufs=MTG, space="PSUM") as psY,    # Y accumulators
            tc.tile_pool(name="psaux", bufs=1, space="PSUM") as psaux,  # small transposes
            tc.tile_pool(name="dram", bufs=1, space="DRAM") as dramp,
        ):
            # ---- constants ----
            identity = constp.tile([P, P], F32)
            make_identity(nc, identity[:])
            identity_bf = constp.tile([P, P], BF16)
            make_identity(nc, identity_bf[:])
            ones_row = constp.tile([1, P], F32)
            nc.gpsimd.memset(ones_row[:], 1.0)
            b_sb = constp.tile([1, D], F32R)
            nc.sync.dma_start(b_sb[:], b_d.ap())
            w_f32 = fstreamp.tile([P, 4 * D], F32, tag="fch")
            for wc in range(4):
                nc.sync.dma_start(
                    w_f32[:, wc * D : (wc + 1) * D], w_d.ap()[wc * P : (wc + 1) * P, :]
                )
            w_sb = constp.tile([P, 4 * D], BF16)  # w chunk wc at [:, wc*D:(wc+1)*D]
            nc.vector.tensor_copy(w_sb[:], w_f32[:])

            # fully resident transposed-A store, fp8:
            # k-chunk kc of m-tile mt at [:, (mt*KC + kc)*P : (mt*KC + kc + 1)*P]
            at_res = atresp.tile([P, MT * KC * P], FP8)
            cin = dramp.tile([MT, P], F32)
            cout = dramp.tile([KC, P], F32)

            def dsq_col(kc):
                return dsq[:, kc : kc + 1]

            dsq = degp.tile([P, KC], F32)

            # ---- merged pass: degree + transpose-all ----
            degree_sb = degp.tile([P, MT], F32)  # col mt = degree of rows mt*128..
            for mt in range(MT):
                dcols = smallp.tile([P, NACH], F32, tag="dcols")
                for c in range(NACH):
                    ach = astreamp.tile([P, ACH], F32, tag="ach")
                    nc.sync.dma_start(
                        ach[:], a_d.ap()[mt * P : (mt + 1) * P, c * ACH : (c + 1) * ACH]
                    )
                    achb = astreamp.tile([P, ACH], BF16, tag="achb", bufs=2)
                    nc.vector.tensor_scalar(
                        achb[:],
                        ach[:],
                        1.0,
                        0.0,
                        op0=ALU.mult,
                        op1=ALU.add,
                        accum_out=dcols[:, c : c + 1],
                    )
                    for g in range(GPC):
                        kc0 = c * GPC * 4 + g * 4  # first k-chunk of this group
                        trp = psA.tile([P, 4 * P], F32, tag="trp")
                        for q in range(4):
                            nc.tensor.matmul(
                                trp[:, q * P : (q + 1) * P],
                                achb[:, (g * 4 + q) * P : (g * 4 + q + 1) * P],
                                identity_bf[:],
                            )
                        dst = at_res[:, (mt * KC + kc0) * P : (mt * KC + kc0 + 4) * P]
                        nc.scalar.activation(dst, trp[:], ACT.Copy)
                nc.vector.reduce_sum(degree_sb[:, mt : mt + 1], dcols[:], axis=AX)

            # ---- AllGather degree (single collective) ----
            aux = psaux.tile([P, 4 * P], F32, tag="aux")
            degT_ps = aux[0:MT, 0:P]
            nc.tensor.transpose(degT_ps, degree_sb[:], identity[:])
            degT_sb = smallp.tile([MT, P], F32, tag="degT")
            nc.vector.tensor_copy(degT_sb[:], degT_ps)
            nc.sync.dma_start(cin[:], degT_sb[:])
            nc.gpsimd.collective_compute(
                "AllGather",
                ALU.bypass,
                ins=[cin.opt()],
                outs=[cout.opt()],
                replica_groups=[list(range(NCORES))],
            )
            # cout row g = degree of global k-chunk g
            degall_sb = smallp.tile([KC, P], F32, tag="degall")
            nc.sync.dma_start(degall_sb[:], cout[:])
            aux2 = psaux.tile([P, 4 * P], F32, tag="aux")
            degallT_ps = aux2[0:P, 0:KC]
            nc.tensor.transpose(degallT_ps, degall_sb[:], identity[:KC, :KC])
            recip = smallp.tile([P, KC], F32, tag="recip")
            nc.vector.reciprocal(recip[:], degallT_ps)
            # dsq[p, g] = 64 * ds[g*128 + p]  (x64 fp8 exponent boost)
            nc.scalar.activation(dsq[:], recip[:], ACT.Sqrt, scale=4096.0)

            # local ds of own rows, /64 to undo the fp8 boost
            recip8 = degp.tile([P, MT], F32)
            nc.vector.reciprocal(recip8[:], degree_sb[:])
            dsown = degp.tile([P, MT], F32)
            nc.scalar.activation(dsown[:], recip8[:], ACT.Sqrt, scale=1.0 / 4096.0)

            # X' = dsq * f in fp8; produced during mtg 0 below.
            xp_sb = xpp.tile([P, KC * D], FP8)  # chunk kc at [:, kc*D:(kc+1)*D]
            # f block fb covers k-chunks 4*fb..4*fb+3
            f_blk = f_d.ap().rearrange("(b c p) d -> b p c d", c=4, p=P)

            # ---- main matmul: fp8 DoubleRow, kc-pair-outer, 2 groups of 4 m-tiles ----
            for mtg in range(MT // MTG):
                # prefetch residual rows for this group's epilogue
                ress = []
                for mi in range(MTG):
                    mt = mtg * MTG + mi
                    res = epip.tile([P, D], F32, tag="res", bufs=MTG)
                    nc.sync.dma_start(res[:], fres_d.ap()[mt * P : (mt + 1) * P, :])
                    ress.append(res)
                ys = [
                    psY.tile([P, D], F32, tag="y", name=f"y{mtg}_{i}")
                    for i in range(MTG)
                ]
                for j in range(PAIRS):
                    jn = j
                    if mtg == 0 and jn % 2 == 0:
                        # stream the f block feeding this pair + the next
                        fb = j // 2
                        fch = fstreamp.tile([P, 4 * D], F32, tag="fch")
                        nc.sync.dma_start(
                            fch[:].rearrange("p (c d) -> p c d", c=4), f_blk[fb]
                        )
                        for t in range(4):
                            kc = 4 * fb + t
                            nc.vector.tensor_scalar_mul(
                                xp_sb[:, kc * D : (kc + 1) * D],
                                fch[:, t * D : (t + 1) * D],
                                dsq_col(kc),
                            )
                    rhs = xp_sb[:, (2 * j) * D : (2 * j + 2) * D].rearrange(
                        "p (two n) -> p two n", two=2
                    )
                    for mi in range(MTG):
                        mt = mtg * MTG + mi
                        lhsT = at_res[
                            :, (mt * KC + 2 * j) * P : (mt * KC + 2 * j + 2) * P
                        ].rearrange("p (two m) -> p two m", two=2)
                        nc.tensor.matmul(
                            ys[mi][:],
                            lhsT,
                            rhs,
                            start=(jn == 0),
                            stop=(jn == PAIRS - 1),
                            perf_mode=DR,
                        )
                # epilogue per m-tile in the group
                for mi in range(MTG):
                    mt = mtg * MTG + mi
                    mf = epip.tile([P, D], BF16, tag="mf")
                    nc.vector.scalar_tensor_tensor(
                        mf[:],
                        ys[mi][:],
                        dsown[:, mt : mt + 1],
                        ress[mi][:],
                        op0=ALU.mult,
                        op1=ALU.add,
                    )
                    o_ps = psA.tile([P, D], F32, tag="trp")
                    aux = psaux.tile([P, 4 * P], F32, tag="aux")
                    for wc in range(4):
                        mfT_ps = aux[:, wc * P : (wc + 1) * P]
                        nc.tensor.matmul(
                            mfT_ps, mf[:, wc * P : (wc + 1) * P], identity_bf[:]
                        )
                        mfT_sb = mftp.tile([P, P], BF16, tag="mfT")
                        nc.scalar.activation(mfT_sb[:], mfT_ps, ACT.Copy)
                        nc.tensor.matmul(
                            o_ps[:],
                            mfT_sb[:],
                            w_sb[:, wc * D : (wc + 1) * D],
                            start=(wc == 0),
                            stop=False,
                        )
                    nc.tensor.matmul(
                        o_ps[:], ones_row[:].bitcast(F32R), b_sb[:],
                        start=False, stop=True,
                    )
                    osb = epip.tile([P, D], F32, tag="osb")
                    nc.scalar.activation(osb[:], o_ps[:], ACT.Relu)
                    nc.sync.dma_start(out_d.ap()[mt * P : (mt + 1) * P, :], osb[:])

    nc.compile()
    return nc


def _get_nc():
    if "nc" not in _NC_CACHE:
        _NC_CACHE["nc"] = _build()
    return _NC_CACHE["nc"]


def run(inputs, trace=False, trace_kwargs=None):
    """Run the SPMD kernel; returns (full_output, BassKernelResults)."""
    a = np.ascontiguousarray(np.asarray(inputs["adjacency_matrix"], dtype=np.float32))
    f = np.ascontiguousarray(np.asarray(inputs["feature"], dtype=np.float32))
    w = np.ascontiguousarray(np.asarray(inputs["W"], dtype=np.float32))
    b = np.ascontiguousarray(np.asarray(inputs["b"], dtype=np.float32)).reshape(1, D)

    nc = _get_nc()
    in_maps = []
    for d in range(NCORES):
        rows = slice(d * R, (d + 1) * R)
        in_maps.append({"a": a[rows], "f": f, "fres": f[rows], "w": w, "bias": b})
    res = bass_utils.run_bass_kernel_spmd(
        nc,
        in_maps,
        core_ids=list(range(NCORES)),
        trace=trace,
        **(trace_kwargs or {}),
    )
    out = np.concatenate([r["out"] for r in res.results], axis=0)
    return out, res


def kernel(**inputs):
    out, _ = run(inputs, trace=False)
    return out


# revision 20
# speedup vs baseline: 1.1383x; 1.0180x over previous
"""GNN message-passing layer (LplsNorm + residual conv) on 8 Trainium2 cores.

Computation (reference, all f32):
    degree = A.sum(-1); ds = degree**-0.5
    mf  = f + ds[:,None] * (A @ (ds[:,None] * f))      # a_norm = ds A ds
    out = relu(mf @ W + b)

Distribution: A row-sharded over 8 cores ([1024, 8192] each), feature
replicated.

Per-core schedule (v5):
  - Single streaming pass over the A shard. Per [128, 2048] f32 chunk:
    DVE casts to bf16 + accumulates exact f32 row sums (degree), TensorE
    transposes the 16 [128,128] bf16 tiles via identity matmuls, ScalarE
    copies them out of PSUM as fp8(e4m3). The ENTIRE transposed shard
    stays SBUF-resident (8 MiB fp8 = 64 KiB/partition) - no DRAM scratch.
  - Degree AllGather is SPLIT in two: half 1 (m-tiles 0-3) is issued
    mid-phase-A and completes in its shadow; half 2 (m-tiles 4-7) is
    issued at phase-A end and hides under the first matmul half-pass,
    which only touches k-chunks whose ds came from half 1.
  - dsq = 64/sqrt(degree) (the x64 fp8 exponent boost is undone in the
    epilogue row scale dsown = sqrt(1/deg)/64).
  - X' = dsq * f in fp8, produced just-in-time from streamed f32 f
    chunks; f blocks are split so each matmul half-pass streams exactly
    the rows it needs.
  - Main matmul in fp8 DoubleRow mode (K=256 per instruction, 2x bf16
    rate), kc-pair-outer over 2 groups of 4 m-tiles (4 PSUM banks).
  - Epilogue per m-tile: mf = Y * dsown + f_res (DVE, bf16 out), mf @ W
    in bf16, bias via a K=1 f32r matmul, ACT relu. PSUM: o accumulator
    shares the phase-A transpose pool (2 bufs) so m-tiles pipeline.
"""

import numpy as np

import concourse.bass as bass
import concourse.mybir as mybir
import concourse.tile as tile
from concourse import bacc
from concourse import bass_utils
from concourse.masks import make_identity

N = 8192
D = 512
NCORES = 8
P = 128
R = N // NCORES          # rows per core: 1024
MT = R // P              # m-tiles per core: 8
KC = N // P              # k-chunks: 64
PAIRS = KC // 2          # DoubleRow k-pairs: 32
ACH = 2048               # A stream chunk width (f32 -> 1 MiB per DMA)
NACH = N // ACH          # stream chunks per row-block: 4
GPC = ACH // (4 * P)     # transpose groups (of 4 tiles) per stream chunk: 4
MTG = 4                  # m-tiles per matmul group (PSUM accumulators)

F32 = mybir.dt.float32
F32R = mybir.dt.float32r
BF16 = mybir.dt.bfloat16
FP8 = mybir.dt.float8e4

_NC_CACHE = {}


def _build():
    nc = bacc.Bacc("TRN2", target_bir_lowering=False, debug=False, num_devices=NCORES)

    a_d = nc.dram_tensor("a", [R, N], F32, kind="ExternalInput")
    f_d = nc.dram_tensor("f", [N, D], F32, kind="ExternalInput")
    fres_d = nc.dram_tensor("fres", [R, D], F32, kind="ExternalInput")
    w_d = nc.dram_tensor("w", [D, D], F32, kind="ExternalInput")
    b_d = nc.dram_tensor("bias", [1, D], F32R, kind="ExternalInput")
    out_d = nc.dram_tensor("out", [R, D], F32, kind="ExternalOutput")

    AX = mybir.AxisListType.X
    ALU = mybir.AluOpType
    ACT = mybir.ActivationFunctionType
    DR = mybir.MatmulPerfMode.DoubleRow

    with tile.TileContext(nc) as tc:
        with (
            tc.tile_pool(name="const", bufs=1) as constp,
            tc.tile_pool(name="deg", bufs=1) as degp,
            tc.tile_pool(name="astream", bufs=4) as astreamp,
            tc.tile_pool(name="small", bufs=2) as smallp,
            tc.tile_pool(name="atres", bufs=1) as atresp,
            tc.tile_pool(name="xp", bufs=1) as xpp,
            tc.tile_pool(name="fstream", bufs=4) as fstreamp,
            tc.tile_pool(name="epi", bufs=2) as epip,
            tc.tile_pool(name="mft", bufs=2) as mftp,
            tc.tile_pool(name="psA", bufs=2, space="PSUM") as psA,      # transposes + o
            tc.tile_pool(name="psY", bufs=MTG, space="PSUM") as psY,    # Y accumulators
            tc.tile_pool(name="psaux", bufs=2, space="PSUM") as psaux,  # small transposes
            tc.tile_pool(name="dram", bufs=1, space="DRAM") as dramp,
        ):
            # ---- constants ----
            identity = constp.tile([P, P], F32)
            make_identity(nc, identity[:])
            identity_bf = constp.tile([P, P], BF16)
            make_identity(nc, identity_bf[:])
            ones_row = constp.tile([1, P], F32)
            nc.gpsimd.memset(ones_row[:], 1.0)
            b_sb = constp.tile([1, D], F32R)
            nc.sync.dma_start(b_sb[:], b_d.ap())
            w_f32 = fstreamp.tile([P, 4 * D], F32, tag="fch")
            for wc in range(4):
                nc.sync.dma_start(
                    w_f32[:, wc * D : (wc + 1) * D], w_d.ap()[wc * P : (wc + 1) * P, :]
                )
            w_sb = constp.tile([P, 4 * D], BF16)  # w chunk wc at [:, wc*D:(wc+1)*D]
            nc.vector.tensor_copy(w_sb[:], w_f32[:])

            # fully resident transposed-A store, fp8:
            # k-chunk kc of m-tile mt at [:, (mt*KC + kc)*P : (mt*KC + kc + 1)*P]
            at_res = atresp.tile([P, MT * KC * P], FP8)
            cin = dramp.tile([MT, P], F32)
            cout = dramp.tile([KC, P], F32)

            def dsq_col(kc):
                return dsq[:, kc : kc + 1]

            dsq = degp.tile([P, KC], F32)

            # ---- merged pass: degree + transpose-all ----
            degree_sb = degp.tile([P, MT], F32)  # col mt = degree of rows mt*128..
            for mt in range(MT):
                dcols = smallp.tile([P, NACH], F32, tag="dcols")
                for c in range(NACH):
                    ach = astreamp.tile([P, ACH], F32, tag="ach")
                    nc.sync.dma_start(
                        ach[:], a_d.ap()[mt * P : (mt + 1) * P, c * ACH : (c + 1) * ACH]
                    )
                    achb = astreamp.tile([P, ACH], BF16, tag="achb", bufs=3)
                    nc.vector.tensor_scalar(
                        achb[:],
                        ach[:],
                        1.0,
                        0.0,
                        op0=ALU.mult,
                        op1=ALU.add,
                        accum_out=dcols[:, c : c + 1],
                    )
                    for g in range(GPC):
                        kc0 = c * GPC * 4 + g * 4  # first k-chunk of this group
                        trp = psA.tile([P, 4 * P], F32, tag="trp")
                        for q in range(4):
                            nc.tensor.matmul(
                                trp[:, q * P : (q + 1) * P],
                                achb[:, (g * 4 + q) * P : (g * 4 + q + 1) * P],
                                identity_bf[:],
                            )
                        dst = at_res[:, (mt * KC + kc0) * P : (mt * KC + kc0 + 4) * P]
                        nc.scalar.activation(dst, trp[:], ACT.Copy)
                nc.vector.reduce_sum(degree_sb[:, mt : mt + 1], dcols[:], axis=AX)

            # ---- AllGather degree (single collective) ----
            aux = psaux.tile([P, 4 * P], F32, tag="aux")
            degT_ps = aux[0:MT, 0:P]
            nc.tensor.transpose(degT_ps, degree_sb[:], identity[:])
            degT_sb = smallp.tile([MT, P], F32, tag="degT")
            nc.vector.tensor_copy(degT_sb[:], degT_ps)
            nc.sync.dma_start(cin[:], degT_sb[:])
            nc.gpsimd.collective_compute(
                "AllGather",
                ALU.bypass,
                ins=[cin.opt()],
                outs=[cout.opt()],
                replica_groups=[list(range(NCORES))],
            )
            # cout row g = degree of global k-chunk g
            degall_sb = smallp.tile([KC, P], F32, tag="degall")
            nc.sync.dma_start(degall_sb[:], cout[:])
            aux2 = psaux.tile([P, 4 * P], F32, tag="aux")
            degallT_ps = aux2[0:P, 0:KC]
            nc.tensor.transpose(degallT_ps, degall_sb[:], identity[:KC, :KC])
            recip = smallp.tile([P, KC], F32, tag="recip")
            nc.vector.reciprocal(recip[:], degallT_ps)
            # dsq[p, g] = 64 * ds[g*128 + p]  (x64 fp8 exponent boost)
            nc.scalar.activation(dsq[:], recip[:], ACT.Sqrt, scale=4096.0)

            # local ds of own rows, /64 to undo the fp8 boost
            recip8 = degp.tile([P, MT], F32)
            nc.vector.reciprocal(recip8[:], degree_sb[:])
            dsown = degp.tile([P, MT], F32)
            nc.scalar.activation(dsown[:], recip8[:], ACT.Sqrt, scale=1.0 / 4096.0)

            # X' = dsq * f in fp8; produced during mtg 0 below.
            xp_sb = xpp.tile([P, KC * D], FP8)  # chunk kc at [:, kc*D:(kc+1)*D]
            # f block fb covers k-chunks 4*fb..4*fb+3
            f_blk = f_d.ap().rearrange("(b c p) d -> b p c d", c=4, p=P)

            # ---- main matmul: fp8 DoubleRow, kc-pair-outer, 2 groups of 4 m-tiles ----
            for mtg in range(MT // MTG):
                # prefetch residual rows for this group's epilogue
                ress = []
                for mi in range(MTG):
                    mt = mtg * MTG + mi
                    res = epip.tile([P, D], F32, tag="res", bufs=MTG)
                    nc.sync.dma_start(res[:], fres_d.ap()[mt * P : (mt + 1) * P, :])
                    ress.append(res)
                ys = [
                    psY.tile([P, D], F32, tag="y", name=f"y{mtg}_{i}")
                    for i in range(MTG)
                ]
                for j in range(PAIRS):
                    jn = j
                    if mtg == 0 and jn % 2 == 0:
                        # stream the f block feeding this pair + the next
                        fb = j // 2
                        fch = fstreamp.tile([P, 4 * D], F32, tag="fch")
                        nc.sync.dma_start(
                            fch[:].rearrange("p (c d) -> p c d", c=4), f_blk[fb]
                        )
                        for t in range(4):
                            kc = 4 * fb + t
                            nc.vector.tensor_scalar_mul(
                                xp_sb[:, kc * D : (kc + 1) * D],
                                fch[:, t * D : (t + 1) * D],
                                dsq_col(kc),
                            )
                    rhs = xp_sb[:, (2 * j) * D : (2 * j + 2) * D].rearrange(
                        "p (two n) -> p two n", two=2
                    )
                    for mi in range(MTG):
                        mt = mtg * MTG + mi
                        lhsT = at_res[
                            :, (mt * KC + 2 * j) * P : (mt * KC + 2 * j + 2) * P
                        ].rearrange("p (two m) -> p two m", two=2)
                        nc.tensor.matmul(
                            ys[mi][:],
                            lhsT,
                            rhs,
                            start=(jn == 0),
                            stop=(jn == PAIRS - 1),
                            perf_mode=DR,
                        )
                # epilogue per m-tile in the group
                for mi in range(MTG):
                    mt = mtg * MTG + mi
                    mf = epip.tile([P, D], BF16, tag="mf")
                    nc.vector.scalar_tensor_tensor(
                        mf[:],
                        ys[mi][:],
                        dsown[:, mt : mt + 1],
                        ress[mi][:],
                        op0=ALU.mult,
                        op1=ALU.add,
                    )
                    o_ps = psA.tile([P, D], F32, tag="trp")
                    # bias first: depends only on constants, so it runs off
                    # the critical mf -> transpose -> matmul chain
                    nc.tensor.matmul(
                        o_ps[:], ones_row[:].bitcast(F32R), b_sb[:],
                        start=True, stop=False,
                    )
                    aux = psaux.tile([P, 4 * P], F32, tag="aux")
                    for wc in range(4):
                        mfT_ps = aux[:, wc * P : (wc + 1) * P]
                        nc.tensor.matmul(
                            mfT_ps, mf[:, wc * P : (wc + 1) * P], identity_bf[:]
                        )
                        mfT_sb = mftp.tile([P, P], BF16, tag="mfT", bufs=4)
                        nc.scalar.activation(mfT_sb[:], mfT_ps, ACT.Copy)
                        nc.tensor.matmul(
                            o_ps[:],
                            mfT_sb[:],
                            w_sb[:, wc * D : (wc + 1) * D],
                            start=False,
                            stop=(wc == 3),
                        )
                    osb = epip.tile([P, D], F32, tag="osb")
                    nc.scalar.activation(osb[:], o_ps[:], ACT.Relu)
                    nc.sync.dma_start(out_d.ap()[mt * P : (mt + 1) * P, :], osb[:])

    nc.compile()
    return nc


def _get_nc():
    if "nc" not in _NC_CACHE:
        _NC_CACHE["nc"] = _build()
    return _NC_CACHE["nc"]


def run(inputs, trace=False, trace_kwargs=None):
    """Run the SPMD kernel; returns (full_output, BassKernelResults)."""
    a = np.ascontiguousarray(np.asarray(inputs["adjacency_matrix"], dtype=np.float32))
    f = np.ascontiguousarray(np.asarray(inputs["feature"], dtype=np.float32))
    w = np.ascontiguousarray(np.asarray(inputs["W"], dtype=np.float32))
    b = np.ascontiguousarray(np.asarray(inputs["b"], dtype=np.float32)).reshape(1, D)

    nc = _get_nc()
    in_maps = []
    for d in range(NCORES):
        rows = slice(d * R, (d + 1) * R)
        in_maps.append({"a": a[rows], "f": f, "fres": f[rows], "w": w, "bias": b})
    res = bass_utils.run_bass_kernel_spmd(
        nc,
        in_maps,
        core_ids=list(range(NCORES)),
        trace=trace,
        **(trace_kwargs or {}),
    )
    out = np.concatenate([r["out"] for r in res.results], axis=0)
    return out, res


def kernel(**inputs):
    out, _ = run(inputs, trace=False)
    return out
